# revision 21
# baseline (speedup 1.0000x reference)
"""Multi-head attention (B=4, N=2048, EMB=768, H=8, D=96) on 8 TRN2 NeuronCores.

Sharding: core c -> batch b = c//2, head group = 4 heads (c%2)*4 .. (c%2)*4+3.
Each core computes the qkv projection for its batch restricted to its heads,
full-sequence attention for those heads, and a partial output projection.
Host sums the two partials per batch and adds b_proj (plus the folded v-bias
term INV_SCALE * bv @ w_proj -- softmax rows sum to exactly INV_SCALE, so the
v-bias contribution to the output is a constant vector).

All matmuls run in float32r (TF32-like, 1 cycle/row at free dim >= 256).
The q/k projections and the output projection pack the 96-dim heads densely
into 128-row stationary tiles (6 q|k blocks instead of 8 per-head ones; 3
proj contraction chunks instead of 4), trading a few extra <=32-partition
PSUM->SBUF copies for a 25-33% cut in projection matmul columns.
Softmax skips the per-row max-subtraction: a global constant SHIFT keeps exp
arguments below ~45, and softmax is invariant to a uniform shift. Row sums
come out of the attn@v matmul via a sqrt(D)-valued extra column in v, so one
DVE reciprocal directly yields INV_SCALE/sum; a rank-1 matmul broadcasts it
and one DVE multiply applies it.
"""
import functools
import math
from contextlib import ExitStack

import numpy as np

import concourse.bass as bass
import concourse.tile as tile
from concourse import bacc, mybir
from concourse.bass_utils import run_bass_kernel_spmd


def _pin_combined_act_table():
    """Make every Exp/Ln activation resolve to one table set that contains
    both, so table swaps don't each pay a ~1.5us ACT_TABLE_LOAD. Table list
    order and length are preserved (the emitted act_func_set_id indexes
    act_info.json), only the membership claimed by the non-combined sets is
    shrunk so the chooser can't pick them."""
    import concourse.hw_specs as hw_specs

    orig = hw_specs.get_activation_tables
    AFT = mybir.ActivationFunctionType

    @functools.cache
    def patched(arch):
        tabs = orig(arch)
        both = {n for n, s in tabs.items() if AFT.Exp in s and AFT.Ln in s}
        if not both:
            return tabs
        return {n: (set(s) if n in both else set(s) - {AFT.Exp, AFT.Ln})
                for n, s in tabs.items()}

    bacc.get_activation_tables = patched
    try:
        import concourse.bass_interp as bass_interp
        bass_interp.get_activation_tables = patched
    except ImportError:
        pass


_pin_combined_act_table()

F32 = mybir.dt.float32
F32R = mybir.dt.float32r
AF = mybir.ActivationFunctionType
ALU = mybir.AluOpType

B, N, EMB, H, D = 4, 2048, 768, 8, 96
HPC = 4                      # heads per core
NCORES = 8
INV_SCALE = 1.0 / math.sqrt(D)
SHIFT = 44.0                 # global exp-argument shift (see module docstring)
EC = EMB // 128              # 6 contraction chunks over emb
IC = N // 128                # 16 token chunks of 128
IB = N // 512                # 4 token blocks of 512
JC = N // 128                # 16 key chunks of 128

# Packed q/k projection: output dims ordered (q0,k0,q1,k1,q2,k2,q3,k3), 96
# rows each, tiled into 6 blocks of 128 PSUM partitions. Each block scatters
# into per-head [D, N] tiles via segments (psum_lo, width, tensor_idx,
# dst_lo); widths/bases obey the >32-partitions-start-at-0 (64 at 0/64) rule.
QKSEG = [
    [(0, 96, 0, 0), (96, 32, 1, 0)],
    [(0, 32, 1, 32), (32, 32, 1, 64), (64, 64, 2, 0)],
    [(0, 32, 2, 64), (32, 32, 3, 0), (64, 32, 3, 32), (96, 32, 3, 64)],
    [(0, 96, 4, 0), (96, 32, 5, 0)],
    [(0, 32, 5, 32), (32, 32, 5, 64), (64, 64, 6, 0)],
    [(0, 32, 6, 64), (32, 32, 7, 0), (64, 32, 7, 32), (96, 32, 7, 64)],
]
# Packed output projection: head h's [D, 512] window block scatters into 3
# [128, N] tiles whose rows follow w_proj's (h, d) row order.
PSEG = [
    [(0, 96, 0, 0)],
    [(0, 32, 0, 96), (32, 32, 1, 0), (64, 32, 1, 32)],
    [(0, 64, 1, 64), (64, 32, 2, 0)],
    [(0, 32, 2, 32), (32, 32, 2, 64), (64, 32, 2, 96)],
]

_cache = {}


def _build(reps=1, dynamic=False):
    nc = bacc.Bacc("TRN2", target_bir_lowering=False, debug=False,
                   num_devices=NCORES)
    xT = nc.dram_tensor("xT", [EMB, N], F32R, kind="ExternalInput").ap()
    wqk = nc.dram_tensor("wqk", [EMB, 2 * HPC * D], F32R, kind="ExternalInput").ap()
    wv = nc.dram_tensor("wv", [EMB, HPC * D], F32R, kind="ExternalInput").ap()
    b12 = nc.dram_tensor("b12", [128, EC], F32, kind="ExternalInput").ap()
    wp = nc.dram_tensor("wp", [HPC * D, EMB], F32R, kind="ExternalInput").ap()
    onesd = nc.dram_tensor("ones", [128, D], F32R, kind="ExternalInput").ap()
    nrep = None
    if dynamic:
        nrep = nc.dram_tensor("nrep", [1, 1], mybir.dt.int32,
                              kind="ExternalInput").ap()
    y = nc.dram_tensor("y", [N, EMB], F32, kind="ExternalOutput").ap()

    with tile.TileContext(nc) as tc, ExitStack() as ctx:
        big = ctx.enter_context(tc.tile_pool(name="big", bufs=24))
        yhp = ctx.enter_context(tc.tile_pool(name="yhp", bufs=3))
        qkp = ctx.enter_context(tc.tile_pool(name="qkp", bufs=6))
        wpool = ctx.enter_context(tc.tile_pool(name="wpool", bufs=6))
        wvp = ctx.enter_context(tc.tile_pool(name="wvp", bufs=6))
        vp = ctx.enter_context(tc.tile_pool(name="vp", bufs=16))
        ep = ctx.enter_context(tc.tile_pool(name="ep", bufs=4))
        ysp = ctx.enter_context(tc.tile_pool(name="ysp", bufs=2))
        sp = ctx.enter_context(tc.tile_pool(name="sp", bufs=1))
        pp = ctx.enter_context(tc.tile_pool(name="pp", bufs=2))
        mmp = ctx.enter_context(tc.tile_pool(name="mmp", bufs=3, space="PSUM"))
        acc = ctx.enter_context(tc.tile_pool(name="acc", bufs=2, space="PSUM"))

        def body():
            # --- load inputs; wqk chunk e rides with x(e, block0) on the
            # same queue so the first projection block unblocks ASAP ---
            wqkt, wvt = [], []
            xt2 = [[None] * IB for _ in range(EC)]
            xeng = [nc.sync, nc.gpsimd, nc.scalar]
            for e in range(EC):
                q = xeng[e % 3]
                t = wpool.tile([128, 2 * HPC * D], F32R, tag="w")
                q.dma_start(out=t[:], in_=wqk[128 * e:128 * (e + 1), :])
                wqkt.append(t)
                t = big.tile([128, 512], F32R, tag="seq")
                q.dma_start(out=t[:], in_=xT[128 * e:128 * (e + 1), 0:512])
                xt2[e][0] = t
            for e in range(EC):
                t = wvp.tile([128, HPC * D], F32R, tag="wv")
                nc.gpsimd.dma_start(out=t[:], in_=wv[128 * e:128 * (e + 1), :])
                wvt.append(t)
            for i4 in range(1, IB):
                for e in range(EC):
                    t = big.tile([128, 512], F32R, tag="seq")
                    xeng[(i4 * EC + e) % 3].dma_start(
                        out=t[:],
                        in_=xT[128 * e:128 * (e + 1),
                               512 * i4:512 * (i4 + 1)])
                    xt2[e][i4] = t
            b12t = sp.tile([128, EC], F32, tag="b12")
            nc.gpsimd.dma_start(out=b12t[:], in_=b12[:])

            onesb = sp.tile([128, D], F32R, tag="onesb")
            nc.gpsimd.dma_start(out=onesb[:], in_=onesd[:])
            ones1 = onesb[0:1, :]
            shiftb = sp.tile([128, 1], F32, tag="shiftb")
            nc.vector.memset(shiftb[:], -SHIFT)
            zerob = sp.tile([1, 1], F32, tag="zerob")
            nc.vector.memset(zerob[:], 0.0)
            # v's sum column is sqrt(D) (not 1) so the av matmul accumulates
            # s/INV_SCALE directly and one reciprocal yields INV_SCALE/s.
            sqrtd = sp.tile([128, HPC], F32, tag="sqrtd")
            nc.vector.memset(sqrtd[:], 1.0 / INV_SCALE)

            # --- v projection groups (emitted inline in head-0 window-0) ---
            vt = [None] * IC

            def v_group(i):
                pv = mmp.tile([128, 512], F32, tag="mm")
                for e in range(EC):
                    nc.tensor.matmul(
                        out=pv[:, :HPC * D],
                        lhsT=xt2[e][i // 4][:, 128 * (i % 4):128 * (i % 4 + 1)],
                        rhs=wvt[e][:],
                        start=(e == 0), stop=(e == EC - 1))
                t = vp.tile([128, HPC, D + 1], F32R, tag="v")
                nc.vector.tensor_copy(
                    out=t[:, :, 0:D],
                    in_=pv[:, :HPC * D].rearrange("p (h d) -> p h d", h=HPC))
                nc.vector.tensor_copy(out=t[:, :, D:D + 1],
                                      in_=sqrtd[:].rearrange(
                                          "p (h o) -> p h o", h=HPC))
                vt[i] = t

            wpt = []
            for t3 in range(3):
                t = wpool.tile([128, EMB], F32R, tag="w")
                nc.scalar.dma_start(out=t[:],
                                    in_=wp[128 * t3:128 * (t3 + 1), :])
                wpt.append(t)

            # Per-head q/k tiles, indexed q0,k0,q1,k1,... Allocated lazily so
            # the 6-buffer ring recycles dead heads' tiles.
            qkT = [None] * (2 * HPC)

            def alloc_qk(idx):
                qkT[idx] = qkp.tile([D, N], F32R, tag="qk",
                                    name=f"qk{idx}")

            def qk_blk(b, i4):
                """One packed q|k projection block [128, 512] -> segments."""
                pq = mmp.tile([128, 512], F32, tag="mm")
                for e in range(EC):
                    nc.tensor.matmul(
                        out=pq[:],
                        lhsT=wqkt[e][:, 128 * b:128 * (b + 1)],
                        rhs=xt2[e][i4][:],
                        start=(e == 0), stop=(e == EC - 1))
                for plo, w, ti, dlo in QKSEG[b]:
                    nc.vector.tensor_scalar(
                        out=qkT[ti][dlo:dlo + w, 512 * i4:512 * (i4 + 1)],
                        in0=pq[plo:plo + w, :],
                        scalar1=b12t[plo:plo + w, b:b + 1], scalar2=None,
                        op0=ALU.add)

            # Packed attention output, rows follow w_proj's (h, d) order.
            Y = [yhp.tile([128, N], F32R, tag="yh", name=f"Y{t3}")
                 for t3 in range(3)]

            def proj_chunk(i):
                """Output projection for token chunk i (needs all Y rows)."""
                ys = ysp.tile([128, EMB], F32, tag="ys")
                for o0, ow in ((0, 512), (512, 256)):
                    py = mmp.tile([128, 512], F32, tag="mm")
                    for t3 in range(3):
                        nc.tensor.matmul(
                            out=py[:, :ow],
                            lhsT=Y[t3][:, 128 * i:128 * (i + 1)],
                            rhs=wpt[t3][:, o0:o0 + ow],
                            start=(t3 == 0), stop=(t3 == 2))
                    nc.vector.tensor_copy(out=ys[:, o0:o0 + ow],
                                          in_=py[:, :ow])
                nc.sync.dma_start(out=y[128 * i:128 * (i + 1), :], in_=ys[:])

            # Filler queue: PE work drained into exp-bound attention windows.
            fillers = []

            def drain(n):
                for _ in range(min(n, len(fillers))):
                    fillers.pop(0)()

            # Deferred-postproc software pipeline: window w's normalizer is
            # finished in window w+1 (DVE reciprocal at w+1 j2==0; PE rank-1
            # broadcast + DVE copy/multiply flushed at w+1 j2==3).
            pending = [None]
            sums_pending = [None]
            lns_pending = [None]
            prev_rec = [None]

            def emit_ln():
                # INV_SCALE/s computed as exp(-ln(s/INV_SCALE)) on the scalar
                # engine: [1,512] single-partition work is ~3x faster there
                # than a DVE reciprocal, and both funcs share one ACT table.
                # The two ops are emitted two iterations apart so each ~0.7us
                # ACT insertion is absorbed by per-iteration ACT slack.
                if sums_pending[0] is None:
                    return
                pavp = sums_pending[0]
                sums_pending[0] = None
                lns = pp.tile([1, 512], F32, tag="lns")
                nc.scalar.activation(out=lns[:], in_=pavp[D:D + 1, :],
                                     func=AF.Ln, bias=zerob[:])
                lns_pending[0] = lns

            def emit_expneg():
                if lns_pending[0] is None:
                    return
                lns = lns_pending[0]
                lns_pending[0] = None
                rec = pp.tile([1, 512], F32R, tag="rec")
                nc.scalar.activation(out=rec[:], in_=lns[:], func=AF.Exp,
                                     scale=-1.0, bias=zerob[:])
                prev_rec[0] = rec

            def flush_pending():
                if pending[0] is not None:
                    pending[0]()
                    pending[0] = None

            for idx in (0, 1, 2):
                alloc_qk(idx)
            qk_blk(0, 0)
            qk_blk(1, 0)

            for h in range(HPC):
                drain(len(fillers))
                if h == 0:
                    alloc_qk(3)
                    fillers.extend([lambda i=i: qk_blk(2, i)
                                    for i in range(IB)])
                elif h == 1:
                    for idx in (4, 5, 6):
                        alloc_qk(idx)
                    fillers.extend(
                        [lambda b=b, i=i: qk_blk(b, i)
                         for i in range(IB) for b in (3, 4)])
                elif h == 2:
                    alloc_qk(7)
                    fillers.extend([lambda i=i: qk_blk(5, i)
                                    for i in range(IB)])
                qt, kt = qkT[2 * h], qkT[2 * h + 1]

                for i4 in range(IB):
                    pav = acc.tile([D + 1, 512], F32, tag="acc")
                    ets = [None] * (JC // 2)

                    def av_pair(p, pav=pav, ets=ets, h=h):
                        for s in range(2):
                            j = 2 * p + s
                            nc.tensor.matmul(
                                out=pav[:], lhsT=vt[j][:, h, :],
                                rhs=ets[p][:, s, :],
                                start=(j == 0), stop=(j == JC - 1))

                    for j2 in range(JC // 2):
                        if h == 0 and i4 == 0:
                            v_group(2 * j2)
                            v_group(2 * j2 + 1)
                            if j2 < 3:
                                qk_blk(0, j2 + 1)
                                qk_blk(1, j2 + 1)
                        ps = mmp.tile([128, 2, 512], F32, tag="mm")
                        for s in range(2):
                            j = 2 * j2 + s
                            nc.tensor.matmul(
                                out=ps[:, s, :],
                                lhsT=kt[:, 128 * j:128 * (j + 1)],
                                rhs=qt[:, 512 * i4:512 * (i4 + 1)],
                                start=True, stop=True)
                        et = ep.tile([128, 2, 512], F32R, tag="e")
                        nc.scalar.activation(
                            out=et[:].rearrange("p a b -> p (a b)"),
                            in_=ps[:].rearrange("p a b -> p (a b)"),
                            func=AF.Exp, bias=shiftb[:])
                        ets[j2] = et
                        if j2 == 1:
                            emit_ln()
                        elif j2 == 3:
                            emit_expneg()
                        if j2 >= 2:
                            av_pair(j2 - 2)
                        if j2 == 4:
                            flush_pending()
                        # Drain fillers front-loaded so the next head's q/k
                        # finish at least one window before its first scores.
                        if j2 in ((1, 5, 7), (1, 2, 5, 7),
                                  (1, 2, 4, 5, 6, 7), (1, 5, 7))[i4] or (
                                h == HPC - 1 and j2 in (2, 4, 6)):
                            drain(1)
                    av_pair(JC // 2 - 2)
                    av_pair(JC // 2 - 1)

                    def post(pav=pav, h=h, i4=i4):
                        rec = prev_rec[0]
                        recb = mmp.tile([128, 512], F32, tag="mm")
                        nc.tensor.matmul(out=recb[:D, :], lhsT=ones1[:],
                                         rhs=rec[:], start=True, stop=True)
                        recs = pp.tile([D, 512], F32, tag="recs")
                        nc.vector.tensor_copy(out=recs[:], in_=recb[:D, :])
                        for plo, w, ti, dlo in PSEG[h]:
                            nc.vector.tensor_tensor(
                                out=Y[ti][dlo:dlo + w,
                                          512 * i4:512 * (i4 + 1)],
                                in0=pav[plo:plo + w, :],
                                in1=recs[plo:plo + w, :], op=ALU.mult)
                        if h == HPC - 1:
                            # final head: queue output projection per block
                            fillers.extend(
                                [lambda i=i: proj_chunk(i)
                                 for i in range(4 * i4, 4 * i4 + 4)])

                    pending[0] = post
                    sums_pending[0] = pav
            emit_ln()
            emit_expneg()
            flush_pending()
            drain(len(fillers))

        if dynamic:
            nt = sp.tile([1, 1], mybir.dt.int32, tag="nrep")
            nc.sync.dma_start(out=nt[:], in_=nrep[:])
            nval = nc.values_load(nt[:], min_val=0, max_val=64)
            with tc.For_i(0, nval, 1):
                body()
        else:
            for _rep in range(reps):
                body()

    nc.compile()
    return nc


def _prep_in_maps(x, w_qkv, b_qkv, w_proj, nrep=None):
    wq = np.ascontiguousarray(w_qkv.reshape(EMB, H, D, 3))
    bq = np.ascontiguousarray(b_qkv.reshape(H, D, 3))
    in_maps = []
    for c in range(NCORES):
        b = c // 2
        h0 = (c % 2) * HPC
        hs = slice(h0, h0 + HPC)
        xTb = np.ascontiguousarray(x[b].T)
        # packed (q0,k0,q1,k1,...) output dims for the q/k projection
        cols, bias = [], []
        for h in range(HPC):
            cols.append(wq[:, h0 + h, :, 0])   # q_h [EMB, D]
            bias.append(bq[h0 + h, :, 0])
            cols.append(wq[:, h0 + h, :, 1])   # k_h
            bias.append(bq[h0 + h, :, 1])
        wqkc = np.concatenate(cols, axis=1)              # [EMB, 768]
        b12c = np.concatenate(bias).reshape(EC, 128).T   # [128, EC]
        wvc = np.ascontiguousarray(wq[:, hs, :, 2].reshape(EMB, HPC * D))
        wpc = np.ascontiguousarray(
            w_proj.reshape(H, D, EMB)[hs].reshape(HPC * D, EMB))
        m = {
            "xT": np.ascontiguousarray(xTb, dtype=np.float32),
            "wqk": np.ascontiguousarray(wqkc, dtype=np.float32),
            "b12": np.ascontiguousarray(b12c, dtype=np.float32),
            "wv": wvc.astype(np.float32, copy=False),
            "wp": wpc.astype(np.float32, copy=False),
            "ones": np.ones((128, D), dtype=np.float32),
        }
        if nrep is not None:
            m["nrep"] = np.array([[nrep]], dtype=np.int32)
        in_maps.append(m)
    return in_maps


def _run(x, w_qkv, b_qkv, w_proj, b_proj, trace=False):
    if "nc" not in _cache:
        _cache["nc"] = _build()
    x = np.asarray(x, dtype=np.float32)
    w_qkv = np.asarray(w_qkv, dtype=np.float32)
    b_qkv = np.asarray(b_qkv, dtype=np.float32)
    w_proj = np.asarray(w_proj, dtype=np.float32)
    in_maps = _prep_in_maps(x, w_qkv, b_qkv, w_proj)
    res = run_bass_kernel_spmd(_cache["nc"], in_maps, list(range(NCORES)),
                               trace=trace)
    # v-bias fold: attn rows sum to exactly INV_SCALE, so the bv term is the
    # constant vector INV_SCALE * (bv @ w_proj); add it host-side with b_proj.
    bv_cat = b_qkv.reshape(H, D, 3)[:, :, 2].reshape(EMB)
    bp = (np.asarray(b_proj, dtype=np.float32)
          + np.float32(INV_SCALE) * (bv_cat @ w_proj))
    out = np.empty((B, N, EMB), dtype=np.float32)
    for b in range(B):
        out[b] = res.results[2 * b]["y"] + res.results[2 * b + 1]["y"] + bp
    return out, res


def kernel(x, w_qkv, b_qkv, w_proj, b_proj):
    out, _ = _run(x, w_qkv, b_qkv, w_proj, b_proj, trace=False)
    return out
